# revision 14
# baseline (speedup 1.0000x reference)
"""Tropical min-max matmul kernel for Trainium2.

out[b, o] = min_i max(x[b, i], weight[i, o])   with  x: [1024, 512], weight: [512, 512], fp32.

Strategy
--------
Data-parallel over the batch dim: 8 NeuronCores x 128 rows of x each; weight
replicated. Per core the i axis (512) is split into 4 partition chunks of 128
and, for each batch row b:

  1. DVE tensor_scalar(max): tmp_t[i', o] = max(w_t[i', o], xT_t[i', b])
     for the 4 i-chunks t. The per-partition scalar operand carries x (the
     host passes x transposed), so no broadcast is needed and fp32
     tensor_scalar runs in the 2x perf mode.
  2. DVE tensor_tensor(min) x2 folds the 4 chunk tiles into M_b[i', o]
     (min over the chunk axis at each of the 128 i' positions).
  3. PE (otherwise idle) transposes the 4 o-blocks of M_b into PSUM.
  4. DVE tensor_reduce(min) over the free axis of the transposed tiles
     finishes the reduction over i' -> out[b, o] for all 512 o in one
     strided-output instruction.

The per-core result lands as ot[128, 4*128] = [o-within-block, block*128+b];
the host reassembles it into out[b, o].
"""

import os
import sys

sys.path.insert(0, "/opt/trn_rl_repo")

import numpy as np

import concourse.bass as bass
import concourse.mybir as mybir
from concourse.bass_utils import run_bass_kernel_spmd

B, I, O = 1024, 512, 512
NCORES = 8
BS = B // NCORES   # 128 batch rows per core
NCH = I // 128     # 4 i-chunks
OBLK = O // 128    # 4 output-feature blocks

# Flipped to True by test.py to collect an NTFF profile; results stashed in
# LAST_RESULTS for inspection.
TRACE = False
LAST_RESULTS = None
# When > 0, kernel() reruns the SPMD executable this many extra times and
# records per-run wall times (seconds) in BENCH_TIMES.
BENCH = 0
BENCH_TIMES = None

_F32 = mybir.dt.float32
_F16 = mybir.dt.float16

# "fp32" (exact) or "fp16" (faster DVE modes, ~1e-4 rel err)
DTYPE_MODE = os.environ.get("MINMAX_DTYPE", "fp32")


def _build_nc(dt, detect_races=True):
    nc = bass.Bass(detect_race_conditions=detect_races)

    xt_d = nc.declare_dram_parameter("xT", [I, BS], dt, isOutput=False)
    w_d = nc.declare_dram_parameter("w", [I, O], dt, isOutput=False)
    id_d = nc.declare_dram_parameter("ident", [128, 128], dt, isOutput=False)
    out_d = nc.declare_dram_parameter("ot", [128, OBLK * BS], dt, isOutput=True)

    with (
        nc.sbuf_tensor([128, NCH * O], dt) as w_sb,     # w chunk t at cols [t*O, (t+1)*O)
        nc.sbuf_tensor([128, NCH * BS], dt) as xt_sb,   # xT chunk t at cols [t*BS, ...)
        nc.sbuf_tensor([128, 128], dt) as id_sb,
        nc.sbuf_tensor([128, 2 * NCH * O], dt) as tmp_sb,   # double-buffered max tiles
        nc.sbuf_tensor([128, 2 * O], dt) as m_sb,           # double-buffered chunk-min
        nc.sbuf_tensor([128, OBLK * BS], dt) as ot_sb,
        nc.psum_tensor([128, O], _F32) as ps0,
        nc.psum_tensor([128, O], _F32) as ps1,
        nc.semaphore("dma_sem") as dma_sem,
        nc.semaphore("tree_sem") as tree_sem,   # DVE: M_b ready
        nc.semaphore("tp_sem") as tp_sem,       # PE: transposes of b done
        nc.semaphore("red_sem") as red_sem,     # DVE: reduce of b done
        nc.Block() as block,
    ):
        ps = [ps0, ps1]
        n_in_dma = 2 * NCH + 1

        @block.sync
        def _(sync):
            for t in range(NCH):
                sync.dma_start(
                    out=w_sb[:, t * O:(t + 1) * O],
                    in_=w_d[t * 128:(t + 1) * 128, :],
                ).then_inc(dma_sem, 16)
                sync.dma_start(
                    out=xt_sb[:, t * BS:(t + 1) * BS],
                    in_=xt_d[t * 128:(t + 1) * 128, :],
                ).then_inc(dma_sem, 16)
            sync.dma_start(out=id_sb[:, :], in_=id_d[:, :]).then_inc(dma_sem, 16)
            sync.wait_ge(red_sem, BS)
            sync.dma_start(out=out_d[:, :], in_=ot_sb[:, :]).then_inc(dma_sem, 16)
            sync.wait_ge(dma_sem, 16 * (n_in_dma + 1))

        @block.tensor
        def _(tensor):
            tensor.wait_ge(dma_sem, 16 * n_in_dma)
            for b in range(BS):
                tensor.wait_ge(tree_sem, b + 1)
                if b >= 2:
                    tensor.wait_ge(red_sem, b - 1)  # psum[b%2] drained
                mb = m_sb[:, (b % 2) * O:(b % 2 + 1) * O]
                for t in range(OBLK):
                    ins = nc.tensor.transpose(
                        ps[b % 2][:, t * 128:(t + 1) * 128],
                        mb[:, t * 128:(t + 1) * 128],
                        id_sb[:, :],
                    )
                ins.then_inc(tp_sem, 1)

        @block.vector
        def _(vector):
            vector.wait_ge(dma_sem, 16 * n_in_dma)
            for b in range(BS):
                j = b % 2
                tb = tmp_sb[:, j * NCH * O:(j + 1) * NCH * O]
                for t in range(NCH):
                    nc.vector.tensor_scalar(
                        out=tb[:, t * O:(t + 1) * O],
                        in0=w_sb[:, t * O:(t + 1) * O],
                        scalar1=xt_sb[:, t * BS + b:t * BS + b + 1],
                        scalar2=None,
                        op0=mybir.AluOpType.max,
                    )
                # fold chunks: (0,2),(1,3) then the remaining pair
                nc.vector.tensor_tensor(
                    out=tb[:, 0:2 * O],
                    in0=tb[:, 0:2 * O],
                    in1=tb[:, 2 * O:4 * O],
                    op=mybir.AluOpType.min,
                )
                if b >= 2:
                    vector.wait_ge(tp_sem, b - 1)  # PE done reading m_sb[b%2]
                nc.vector.tensor_tensor(
                    out=m_sb[:, j * O:(j + 1) * O],
                    in0=tb[:, 0:O],
                    in1=tb[:, O:2 * O],
                    op=mybir.AluOpType.min,
                ).then_inc(tree_sem, 1)
                # finish row b-1: reduce its transposed tiles over i'
                if b >= 1:
                    _reduce_row(nc, vector, ps, ot_sb, b - 1, tp_sem, red_sem)
            _reduce_row(nc, vector, ps, ot_sb, BS - 1, tp_sem, red_sem)

    return nc


def _reduce_row(nc, vector, ps, ot_sb, br, tp_sem, red_sem):
    vector.wait_ge(tp_sem, br + 1)
    src = ps[br % 2][:, :].rearrange("p (t i) -> p t i", t=OBLK)
    dst = ot_sb[:, br::BS]  # columns t*BS + br for t = 0..3
    nc.vector.tensor_reduce(
        out=dst,
        in_=src,
        op=mybir.AluOpType.min,
        axis=mybir.AxisListType.X,
    ).then_inc(red_sem, 1)


_NC_CACHE = {}


def _get_nc(mode):
    if mode not in _NC_CACHE:
        _NC_CACHE[mode] = _build_nc(_F16 if mode == "fp16" else _F32)
    return _NC_CACHE[mode]


def kernel(x, weight):
    global LAST_RESULTS
    x = np.asarray(x)
    weight = np.asarray(weight)
    in_dtype = x.dtype

    mode = DTYPE_MODE
    npdt = np.float16 if mode == "fp16" else np.float32
    nc = _get_nc(mode)

    w_h = np.ascontiguousarray(weight.astype(npdt))
    ident = np.eye(128, dtype=npdt)
    xt_full = np.ascontiguousarray(x.astype(npdt).T)  # [I, B]
    in_maps = [
        {
            "xT": np.ascontiguousarray(xt_full[:, c * BS:(c + 1) * BS]),
            "w": w_h,
            "ident": ident,
        }
        for c in range(NCORES)
    ]

    res = run_bass_kernel_spmd(nc, in_maps, list(range(NCORES)), trace=TRACE)
    LAST_RESULTS = res

    if BENCH > 0:
        import time as _time

        global BENCH_TIMES
        BENCH_TIMES = []
        for _ in range(BENCH):
            t0 = _time.perf_counter()
            run_bass_kernel_spmd(nc, in_maps, list(range(NCORES)), trace=False)
            BENCH_TIMES.append(_time.perf_counter() - t0)

    # ot[oo, t*BS + b] = out_core[b, t*128 + oo]
    parts = []
    for c in range(NCORES):
        ot = np.asarray(res.results[c]["ot"])          # [128, OBLK*BS]
        oc = ot.reshape(128, OBLK, BS).transpose(2, 1, 0).reshape(BS, O)
        parts.append(oc)
    out = np.concatenate(parts, axis=0)
    return out.astype(in_dtype)


# revision 26
# speedup vs baseline: 1.0977x; 1.0977x over previous
"""Tropical min-max matmul kernel for Trainium2.

out[b, o] = min_i max(x[b, i], weight[i, o])   with  x: [1024, 512], weight: [512, 512], fp32.

Strategy
--------
Data-parallel over the batch dim: 8 NeuronCores x 128 rows of x each; weight
replicated. Per core the i axis (512) is split into 4 partition chunks of 128
and, for each batch row b:

  1. DVE tensor_scalar(max): tmp_t[i', o] = max(w_t[i', o], xT_t[i', b])
     for the 4 i-chunks t. The per-partition scalar operand carries x (the
     host passes x transposed), so no broadcast is needed and fp32
     tensor_scalar runs in the 2x perf mode.
  2. DVE tensor_tensor(min) x2 folds the 4 chunk tiles into M_b[i', o]
     (min over the chunk axis at each of the 128 i' positions).
  3. PE (otherwise idle) transposes the 4 o-blocks of M_b into PSUM.
  4. DVE tensor_reduce(min) over the free axis of the transposed tiles
     finishes the reduction over i' -> out[b, o] for all 512 o in one
     strided-output instruction.

The per-core result lands as ot[128, 4*128] = [o-within-block, block*128+b];
the host reassembles it into out[b, o].
"""

import os
import sys

sys.path.insert(0, "/opt/trn_rl_repo")

import numpy as np

import concourse.bass as bass
import concourse.mybir as mybir
from concourse.bass_utils import run_bass_kernel_spmd

B, I, O = 1024, 512, 512
NCORES = 8
BS = B // NCORES   # 128 batch rows per core
NCH = I // 128     # 4 i-chunks
OBLK = O // 128    # 4 output-feature blocks

# Flipped to True by test.py to collect an NTFF profile; results stashed in
# LAST_RESULTS for inspection.
TRACE = False
LAST_RESULTS = None
# When > 0, kernel() reruns the SPMD executable this many extra times and
# records per-run wall times (seconds) in BENCH_TIMES.
BENCH = 0
BENCH_TIMES = None

_F32 = mybir.dt.float32
_F16 = mybir.dt.float16

# "fp32" (exact) or "fp16" (faster DVE modes, ~1e-4 rel err)
DTYPE_MODE = os.environ.get("MINMAX_DTYPE", "fp32")


def _build_nc(dt, detect_races=True, repeat=1):
    nc = bass.Bass(detect_race_conditions=detect_races)
    NB = repeat * BS  # total pipeline iterations (repeat > 1 only for timing)

    xt_d = nc.declare_dram_parameter("xT", [I, BS], dt, isOutput=False)
    w_d = nc.declare_dram_parameter("w", [I, O], dt, isOutput=False)
    id_d = nc.declare_dram_parameter("ident", [128, 128], dt, isOutput=False)
    out_d = nc.declare_dram_parameter("ot", [128, OBLK * BS], dt, isOutput=True)

    with (
        nc.sbuf_tensor([128, NCH * O], dt) as w_sb,     # w chunk t at cols [t*O, (t+1)*O)
        nc.sbuf_tensor([128, NCH * BS], dt) as xt_sb,   # xT chunk t at cols [t*BS, ...)
        nc.sbuf_tensor([128, 128], dt) as id_sb,
        nc.sbuf_tensor([128, 2 * NCH * O], dt) as tmp_sb,   # double-buffered max tiles
        nc.sbuf_tensor([128, 2 * O], dt) as m_sb,           # double-buffered chunk-min
        nc.sbuf_tensor([128, OBLK * BS], dt) as ot_sb,
        nc.psum_tensor([128, O], _F32) as ps0,
        nc.psum_tensor([128, O], _F32) as ps1,
        nc.semaphore("dma_sem") as dma_sem,
        nc.semaphore("tree_sem") as tree_sem,   # DVE: M_b ready
        nc.semaphore("tp_sem") as tp_sem,       # PE: transposes of b done
        nc.semaphore("red_sem") as red_sem,     # DVE: reduce of b done
        nc.Block() as block,
    ):
        ps = [ps0, ps1]
        n_in_dma = 2 * NCH + 1

        @block.sync
        def _(sync):
            for t in range(NCH):
                sync.dma_start(
                    out=w_sb[:, t * O:(t + 1) * O],
                    in_=w_d[t * 128:(t + 1) * 128, :],
                ).then_inc(dma_sem, 16)
                sync.dma_start(
                    out=xt_sb[:, t * BS:(t + 1) * BS],
                    in_=xt_d[t * 128:(t + 1) * 128, :],
                ).then_inc(dma_sem, 16)
            sync.dma_start(out=id_sb[:, :], in_=id_d[:, :]).then_inc(dma_sem, 16)
            sync.wait_ge(red_sem, NB)
            sync.dma_start(out=out_d[:, :], in_=ot_sb[:, :]).then_inc(dma_sem, 16)
            sync.wait_ge(dma_sem, 16 * (n_in_dma + 1))

        @block.tensor
        def _(tensor):
            tensor.wait_ge(dma_sem, 16 * n_in_dma)
            for b in range(NB):
                tensor.wait_ge(tree_sem, b + 1)
                if b >= 2:
                    tensor.wait_ge(red_sem, b - 1)  # psum[b%2] drained
                mb = m_sb[:, (b % 2) * O:(b % 2 + 1) * O]
                for t in range(OBLK):
                    ins = nc.tensor.transpose(
                        ps[b % 2][:, t * 128:(t + 1) * 128],
                        mb[:, t * 128:(t + 1) * 128],
                        id_sb[:, :],
                    )
                ins.then_inc(tp_sem, 1)

        @block.vector
        def _(vector):
            vector.wait_ge(dma_sem, 16 * n_in_dma)
            for b in range(NB):
                j = b % 2
                tb = tmp_sb[:, j * NCH * O:(j + 1) * NCH * O]
                bb = b % BS
                for t in range(NCH):
                    nc.vector.tensor_scalar(
                        out=tb[:, t * O:(t + 1) * O],
                        in0=w_sb[:, t * O:(t + 1) * O],
                        scalar1=xt_sb[:, t * BS + bb:t * BS + bb + 1],
                        scalar2=None,
                        op0=mybir.AluOpType.max,
                    )
                # fold chunks: (0,2),(1,3) then the remaining pair
                nc.vector.tensor_tensor(
                    out=tb[:, 0:2 * O],
                    in0=tb[:, 0:2 * O],
                    in1=tb[:, 2 * O:4 * O],
                    op=mybir.AluOpType.min,
                )
                if b >= 2:
                    vector.wait_ge(tp_sem, b - 1)  # PE done reading m_sb[b%2]
                nc.vector.tensor_tensor(
                    out=m_sb[:, j * O:(j + 1) * O],
                    in0=tb[:, 0:O],
                    in1=tb[:, O:2 * O],
                    op=mybir.AluOpType.min,
                ).then_inc(tree_sem, 1)
                # finish row b-1: reduce its transposed tiles over i'
                if b >= 1:
                    _reduce_row(nc, vector, ps, ot_sb, b - 1, tp_sem, red_sem)
            _reduce_row(nc, vector, ps, ot_sb, NB - 1, tp_sem, red_sem)

    return nc


def _reduce_row(nc, vector, ps, ot_sb, br, tp_sem, red_sem):
    vector.wait_ge(tp_sem, br + 1)
    src = ps[br % 2][:, :].rearrange("p (t i) -> p t i", t=OBLK)
    dst = ot_sb[:, (br % BS)::BS]  # columns t*BS + b for t = 0..3
    nc.vector.tensor_reduce(
        out=dst,
        in_=src,
        op=mybir.AluOpType.min,
        axis=mybir.AxisListType.X,
    ).then_inc(red_sem, 1)


def _build_nc_wide(dt, detect_races=True, repeat=1, group=8):
    """Wide-group variant: GROUP batch rows per step, three fat instructions
    per group (DMA partition-broadcast of x rows; one wide tensor_tensor max
    over [128, GROUP*4*512]; one strided tensor_reduce min). Minimizes
    instruction count while staying near the DVE roofline.

    Needs wT = weight.T from the host: tiles wT_t[o', i] put o' on partitions
    so the i axis is free (reduce axis); x rows broadcast across partitions.
    """
    nc = bass.Bass(detect_race_conditions=detect_races)
    G = group
    NGRP = BS // G
    W = G * NCH * I  # wide op free size per group

    xd = nc.declare_dram_parameter("x", [BS, I], dt, isOutput=False)
    wt_d = nc.declare_dram_parameter("wT", [O, I], dt, isOutput=False)
    out_d = nc.declare_dram_parameter("ot", [128, OBLK * BS], dt, isOutput=True)

    x_rows = xd.rearrange("(g r) i -> g (r i)", r=G)  # [NGRP, G*I]

    with (
        nc.sbuf_tensor([128, OBLK * I], dt) as wt_sb,
        nc.sbuf_tensor([128, 2 * G * I], dt) as bc_sb,   # double-buffered bcast
        nc.sbuf_tensor([128, W], dt) as scr_sb,
        nc.sbuf_tensor([128, OBLK * BS], dt) as ot_sb,
        nc.semaphore("dma_sem") as dma_sem,
        nc.semaphore("v_sem") as v_sem,
        nc.Block() as block,
    ):
        NB = repeat * NGRP

        def bc_tile(g):
            j = g % 2
            return bc_sb[:, j * G * I:(j + 1) * G * I]

        @block.sync
        def _(sync):
            sync.dma_start(
                out=wt_sb[:, :].rearrange("p (t i) -> p t i", t=OBLK),
                in_=wt_d.rearrange("(t p) i -> p t i", p=128),
            ).then_inc(dma_sem, 16)
            for gg in range(NB):
                g = gg % NGRP
                if gg >= 2:
                    sync.wait_ge(v_sem, 2 * gg - 3)  # bc[gg%2] consumed by TT of gg-2
                dst = bc_tile(gg)
                src = x_rows[g:g + 1, :]
                src_b = bass.AP(
                    tensor=src.tensor,
                    offset=src.offset,
                    ap=[[0, 128]] + [list(d) for d in src.ap][1:],
                )
                sync.dma_start(out=dst, in_=src_b).then_inc(dma_sem, 16)
            sync.wait_ge(v_sem, 2 * NB)
            sync.dma_start(out=out_d[:, :], in_=ot_sb[:, :]).then_inc(dma_sem, 16)
            sync.wait_ge(dma_sem, 16 * (NB + 2))

        @block.vector
        def _(vector):
            wt_v = wt_sb[:, :]
            scr_v = scr_sb[:, :]
            for gg in range(NB):
                g = gg % NGRP
                vector.wait_ge(dma_sem, 16 * (gg + 2))
                bc = bc_tile(gg)
                in0 = bass.AP(
                    tensor=wt_v.tensor, offset=wt_v.offset,
                    ap=[[wt_v.ap[0][0], 128], [0, G], [I, OBLK], [1, I]],
                )
                in1 = bass.AP(
                    tensor=bc.tensor, offset=bc.offset,
                    ap=[[bc.ap[0][0], 128], [I, G], [0, OBLK], [1, I]],
                )
                out = bass.AP(
                    tensor=scr_v.tensor, offset=scr_v.offset,
                    ap=[[scr_v.ap[0][0], 128], [OBLK * I, G], [I, OBLK], [1, I]],
                )
                nc.vector.tensor_tensor(
                    out=out, in0=in0, in1=in1, op=mybir.AluOpType.max
                ).then_inc(v_sem, 1)
                ot_ap = ot_sb[:, :]
                red_out = bass.AP(
                    tensor=ot_ap.tensor,
                    offset=ot_ap.offset + g * G,
                    ap=[[ot_ap.ap[0][0], 128], [1, G], [BS, OBLK]],
                )
                nc.vector.tensor_reduce(
                    out=red_out,
                    in_=out,
                    op=mybir.AluOpType.min,
                    axis=mybir.AxisListType.X,
                ).then_inc(v_sem, 1)

    return nc


_NC_CACHE = {}


def _get_nc(mode):
    if mode not in _NC_CACHE:
        _NC_CACHE[mode] = _build_nc_wide(_F16 if mode == "fp16" else _F32)
    return _NC_CACHE[mode]


def kernel(x, weight):
    global LAST_RESULTS
    x = np.asarray(x)
    weight = np.asarray(weight)
    in_dtype = x.dtype

    mode = DTYPE_MODE
    npdt = np.float16 if mode == "fp16" else np.float32
    nc = _get_nc(mode)

    wt_h = np.ascontiguousarray(weight.T.astype(npdt))  # [O, I]
    xh = x.astype(npdt)
    in_maps = [
        {
            "x": np.ascontiguousarray(xh[c * BS:(c + 1) * BS]),
            "wT": wt_h,
        }
        for c in range(NCORES)
    ]

    res = run_bass_kernel_spmd(nc, in_maps, list(range(NCORES)), trace=TRACE)
    LAST_RESULTS = res

    if BENCH > 0:
        import time as _time

        global BENCH_TIMES
        BENCH_TIMES = []
        for _ in range(BENCH):
            t0 = _time.perf_counter()
            run_bass_kernel_spmd(nc, in_maps, list(range(NCORES)), trace=False)
            BENCH_TIMES.append(_time.perf_counter() - t0)

    # ot[oo, t*BS + b] = out_core[b, t*128 + oo]
    parts = []
    for c in range(NCORES):
        ot = np.asarray(res.results[c]["ot"])          # [128, OBLK*BS]
        oc = ot.reshape(128, OBLK, BS).transpose(2, 1, 0).reshape(BS, O)
        parts.append(oc)
    out = np.concatenate(parts, axis=0)
    return out.astype(in_dtype)


# revision 32
# speedup vs baseline: 261.3744x; 238.1097x over previous
"""Tropical min-max matmul kernel for Trainium2.

out[b, o] = min_i max(x[b, i], weight[i, o])   with  x: [1024, 512], weight: [512, 512], fp32.

Strategy
--------
Data-parallel over the batch dim: 8 NeuronCores x 128 rows of x each; weight
replicated. Per core the i axis (512) is split into 4 partition chunks of 128
and, for each batch row b:

  1. DVE tensor_scalar(max): tmp_t[i', o] = max(w_t[i', o], xT_t[i', b])
     for the 4 i-chunks t. The per-partition scalar operand carries x (the
     host passes x transposed), so no broadcast is needed and fp32
     tensor_scalar runs in the 2x perf mode.
  2. DVE tensor_tensor(min) x2 folds the 4 chunk tiles into M_b[i', o]
     (min over the chunk axis at each of the 128 i' positions).
  3. PE (otherwise idle) transposes the 4 o-blocks of M_b into PSUM.
  4. DVE tensor_reduce(min) over the free axis of the transposed tiles
     finishes the reduction over i' -> out[b, o] for all 512 o in one
     strided-output instruction.

The per-core result lands as ot[128, 4*128] = [o-within-block, block*128+b];
the host reassembles it into out[b, o].
"""

import os
import sys

sys.path.insert(0, "/opt/trn_rl_repo")

import numpy as np

import concourse.bass as bass
import concourse.mybir as mybir
from concourse.bass_utils import run_bass_kernel_spmd

B, I, O = 1024, 512, 512
NCORES = 8
BS = B // NCORES   # 128 batch rows per core
NCH = I // 128     # 4 i-chunks
OBLK = O // 128    # 4 output-feature blocks

# Flipped to True by test.py to collect an NTFF profile; results stashed in
# LAST_RESULTS for inspection.
TRACE = False
LAST_RESULTS = None
# When > 0, kernel() reruns the SPMD executable this many extra times and
# records per-run wall times (seconds) in BENCH_TIMES.
BENCH = 0
BENCH_TIMES = None

_F32 = mybir.dt.float32
_F16 = mybir.dt.float16

# "fp32" (exact) or "fp16" (faster DVE modes, ~1e-4 rel err)
DTYPE_MODE = os.environ.get("MINMAX_DTYPE", "fp32")


def _build_nc(dt, detect_races=True, repeat=1):
    nc = bass.Bass(detect_race_conditions=detect_races)
    NB = repeat * BS  # total pipeline iterations (repeat > 1 only for timing)

    xt_d = nc.declare_dram_parameter("xT", [I, BS], dt, isOutput=False)
    w_d = nc.declare_dram_parameter("w", [I, O], dt, isOutput=False)
    id_d = nc.declare_dram_parameter("ident", [128, 128], dt, isOutput=False)
    out_d = nc.declare_dram_parameter("ot", [128, OBLK * BS], dt, isOutput=True)

    with (
        nc.sbuf_tensor([128, NCH * O], dt) as w_sb,     # w chunk t at cols [t*O, (t+1)*O)
        nc.sbuf_tensor([128, NCH * BS], dt) as xt_sb,   # xT chunk t at cols [t*BS, ...)
        nc.sbuf_tensor([128, 128], dt) as id_sb,
        nc.sbuf_tensor([128, 2 * NCH * O], dt) as tmp_sb,   # double-buffered max tiles
        nc.sbuf_tensor([128, 2 * O], dt) as m_sb,           # double-buffered chunk-min
        nc.sbuf_tensor([128, OBLK * BS], dt) as ot_sb,
        nc.psum_tensor([128, O], _F32) as ps0,
        nc.psum_tensor([128, O], _F32) as ps1,
        nc.semaphore("dma_sem") as dma_sem,
        nc.semaphore("tree_sem") as tree_sem,   # DVE: M_b ready
        nc.semaphore("tp_sem") as tp_sem,       # PE: transposes of b done
        nc.semaphore("red_sem") as red_sem,     # DVE: reduce of b done
        nc.Block() as block,
    ):
        ps = [ps0, ps1]
        n_in_dma = 2 * NCH + 1

        @block.sync
        def _(sync):
            for t in range(NCH):
                sync.dma_start(
                    out=w_sb[:, t * O:(t + 1) * O],
                    in_=w_d[t * 128:(t + 1) * 128, :],
                ).then_inc(dma_sem, 16)
                sync.dma_start(
                    out=xt_sb[:, t * BS:(t + 1) * BS],
                    in_=xt_d[t * 128:(t + 1) * 128, :],
                ).then_inc(dma_sem, 16)
            sync.dma_start(out=id_sb[:, :], in_=id_d[:, :]).then_inc(dma_sem, 16)
            sync.wait_ge(red_sem, NB)
            sync.dma_start(out=out_d[:, :], in_=ot_sb[:, :]).then_inc(dma_sem, 16)
            sync.wait_ge(dma_sem, 16 * (n_in_dma + 1))

        @block.tensor
        def _(tensor):
            tensor.wait_ge(dma_sem, 16 * n_in_dma)
            for b in range(NB):
                tensor.wait_ge(tree_sem, b + 1)
                if b >= 2:
                    tensor.wait_ge(red_sem, b - 1)  # psum[b%2] drained
                mb = m_sb[:, (b % 2) * O:(b % 2 + 1) * O]
                for t in range(OBLK):
                    ins = nc.tensor.transpose(
                        ps[b % 2][:, t * 128:(t + 1) * 128],
                        mb[:, t * 128:(t + 1) * 128],
                        id_sb[:, :],
                    )
                ins.then_inc(tp_sem, 1)

        @block.vector
        def _(vector):
            vector.wait_ge(dma_sem, 16 * n_in_dma)
            for b in range(NB):
                j = b % 2
                tb = tmp_sb[:, j * NCH * O:(j + 1) * NCH * O]
                bb = b % BS
                for t in range(NCH):
                    nc.vector.tensor_scalar(
                        out=tb[:, t * O:(t + 1) * O],
                        in0=w_sb[:, t * O:(t + 1) * O],
                        scalar1=xt_sb[:, t * BS + bb:t * BS + bb + 1],
                        scalar2=None,
                        op0=mybir.AluOpType.max,
                    )
                # fold chunks: (0,2),(1,3) then the remaining pair
                nc.vector.tensor_tensor(
                    out=tb[:, 0:2 * O],
                    in0=tb[:, 0:2 * O],
                    in1=tb[:, 2 * O:4 * O],
                    op=mybir.AluOpType.min,
                )
                if b >= 2:
                    vector.wait_ge(tp_sem, b - 1)  # PE done reading m_sb[b%2]
                nc.vector.tensor_tensor(
                    out=m_sb[:, j * O:(j + 1) * O],
                    in0=tb[:, 0:O],
                    in1=tb[:, O:2 * O],
                    op=mybir.AluOpType.min,
                ).then_inc(tree_sem, 1)
                # finish row b-1: reduce its transposed tiles over i'
                if b >= 1:
                    _reduce_row(nc, vector, ps, ot_sb, b - 1, tp_sem, red_sem)
            _reduce_row(nc, vector, ps, ot_sb, NB - 1, tp_sem, red_sem)

    return nc


def _reduce_row(nc, vector, ps, ot_sb, br, tp_sem, red_sem):
    vector.wait_ge(tp_sem, br + 1)
    src = ps[br % 2][:, :].rearrange("p (t i) -> p t i", t=OBLK)
    dst = ot_sb[:, (br % BS)::BS]  # columns t*BS + b for t = 0..3
    nc.vector.tensor_reduce(
        out=dst,
        in_=src,
        op=mybir.AluOpType.min,
        axis=mybir.AxisListType.X,
    ).then_inc(red_sem, 1)


def _build_nc_wide(dt, detect_races=True, repeat=1, group=16):
    """Wide-group variant: GROUP batch rows per step, three fat instructions
    per group (DMA partition-broadcast of x rows; one wide tensor_tensor max
    over [128, GROUP*4*512]; one strided tensor_reduce min). Minimizes
    instruction count while staying near the DVE roofline.

    Needs wT = weight.T from the host: tiles wT_t[o', i] put o' on partitions
    so the i axis is free (reduce axis); x rows broadcast across partitions.
    """
    nc = bass.Bass(detect_race_conditions=detect_races)
    G = group
    NGRP = BS // G
    W = G * NCH * I  # wide op free size per group

    xd = nc.declare_dram_parameter("x", [BS, I], dt, isOutput=False)
    wt_d = nc.declare_dram_parameter("wT", [O, I], dt, isOutput=False)
    out_d = nc.declare_dram_parameter("ot", [128, OBLK * BS], dt, isOutput=True)

    x_rows = xd.rearrange("(g r) i -> g (r i)", r=G)  # [NGRP, G*I]

    with (
        nc.sbuf_tensor([128, OBLK * I], dt) as wt_sb,
        nc.sbuf_tensor([128, 2 * G * I], dt) as bc_sb,   # double-buffered bcast
        nc.sbuf_tensor([128, W], dt) as scr_sb,
        nc.sbuf_tensor([128, OBLK * BS], dt) as ot_sb,
        nc.semaphore("dma_sem") as dma_sem,
        nc.semaphore("v_sem") as v_sem,
        nc.Block() as block,
    ):
        NB = repeat * NGRP

        def bc_tile(g):
            j = g % 2
            return bc_sb[:, j * G * I:(j + 1) * G * I]

        @block.sync
        def _(sync):
            sync.dma_start(
                out=wt_sb[:, :].rearrange("p (t i) -> p t i", t=OBLK),
                in_=wt_d.rearrange("(t p) i -> p t i", p=128),
            ).then_inc(dma_sem, 16)
            # broadcast x rows two groups at a time (one DMA fills both
            # halves of the double buffer)
            n_pair_dma = 0
            for gg in range(0, NB, 2):
                g = gg % NGRP
                if gg >= 2:
                    # both halves consumed by the TTs of gg-2 and gg-1
                    sync.wait_ge(v_sem, 2 * gg - 1)
                src = x_rows[g:g + 2, :]
                src_b = bass.AP(
                    tensor=src.tensor,
                    offset=src.offset,
                    ap=[[0, 128], [G * I, 2], [1, G * I]],
                )
                sync.dma_start(out=bc_sb[:, :], in_=src_b).then_inc(dma_sem, 16)
                n_pair_dma += 1
            sync.wait_ge(v_sem, 2 * NB)
            sync.dma_start(out=out_d[:, :], in_=ot_sb[:, :]).then_inc(dma_sem, 16)
            sync.wait_ge(dma_sem, 16 * (n_pair_dma + 2))

        @block.vector
        def _(vector):
            wt_v = wt_sb[:, :]
            scr_v = scr_sb[:, :]
            for gg in range(NB):
                g = gg % NGRP
                if gg % 2 == 0:
                    vector.wait_ge(dma_sem, 16 * (gg // 2 + 2))
                bc = bc_tile(gg)
                in0 = bass.AP(
                    tensor=wt_v.tensor, offset=wt_v.offset,
                    ap=[[wt_v.ap[0][0], 128], [0, G], [I, OBLK], [1, I]],
                )
                in1 = bass.AP(
                    tensor=bc.tensor, offset=bc.offset,
                    ap=[[bc.ap[0][0], 128], [I, G], [0, OBLK], [1, I]],
                )
                out = bass.AP(
                    tensor=scr_v.tensor, offset=scr_v.offset,
                    ap=[[scr_v.ap[0][0], 128], [OBLK * I, G], [I, OBLK], [1, I]],
                )
                nc.vector.tensor_tensor(
                    out=out, in0=in0, in1=in1, op=mybir.AluOpType.max
                ).then_inc(v_sem, 1)
                ot_ap = ot_sb[:, :]
                red_out = bass.AP(
                    tensor=ot_ap.tensor,
                    offset=ot_ap.offset + g * G,
                    ap=[[ot_ap.ap[0][0], 128], [1, G], [BS, OBLK]],
                )
                nc.vector.tensor_reduce(
                    out=red_out,
                    in_=out,
                    op=mybir.AluOpType.min,
                    axis=mybir.AxisListType.X,
                ).then_inc(v_sem, 1)

    return nc


_NC_CACHE = {}


def _get_nc(mode):
    if mode not in _NC_CACHE:
        if mode == "fp16":
            _NC_CACHE[mode] = _build_nc_wide(_F16, group=16)
        else:
            _NC_CACHE[mode] = _build_nc_wide(_F32, group=16)
    return _NC_CACHE[mode]


def kernel(x, weight):
    global LAST_RESULTS
    x = np.asarray(x)
    weight = np.asarray(weight)
    in_dtype = x.dtype

    mode = DTYPE_MODE
    npdt = np.float16 if mode == "fp16" else np.float32
    nc = _get_nc(mode)

    wt_h = np.ascontiguousarray(weight.T.astype(npdt))  # [O, I]
    xh = x.astype(npdt)
    in_maps = [
        {
            "x": np.ascontiguousarray(xh[c * BS:(c + 1) * BS]),
            "wT": wt_h,
        }
        for c in range(NCORES)
    ]

    res = run_bass_kernel_spmd(nc, in_maps, list(range(NCORES)), trace=TRACE)
    LAST_RESULTS = res

    if BENCH > 0:
        import time as _time

        global BENCH_TIMES
        BENCH_TIMES = []
        for _ in range(BENCH):
            t0 = _time.perf_counter()
            run_bass_kernel_spmd(nc, in_maps, list(range(NCORES)), trace=False)
            BENCH_TIMES.append(_time.perf_counter() - t0)

    # ot[oo, t*BS + b] = out_core[b, t*128 + oo]
    parts = []
    for c in range(NCORES):
        ot = np.asarray(res.results[c]["ot"])          # [128, OBLK*BS]
        oc = ot.reshape(128, OBLK, BS).transpose(2, 1, 0).reshape(BS, O)
        parts.append(oc)
    out = np.concatenate(parts, axis=0)
    return out.astype(in_dtype)


# revision 34
# speedup vs baseline: 340.8117x; 1.3039x over previous
"""Tropical min-max matmul kernel for Trainium2.

out[b, o] = min_i max(x[b, i], weight[i, o])   with  x: [1024, 512], weight: [512, 512], fp32.

Strategy
--------
Data-parallel over the batch dim: 8 NeuronCores x 128 rows of x each; weight
replicated (no collectives). Per core, the weight is held transposed
(wT[o, i], o on partitions in 4 row-blocks) so the contraction axis i is the
DVE free axis, and batch rows are processed in groups of 16 with three fat
instructions per group:

  1. A DMA whose source access pattern has partition stride 0 broadcasts the
     group's x rows across all 128 partitions (SBUF bc tile, double-buffered,
     two groups loaded per DMA).
  2. One wide DVE tensor_tensor(max) over [128, 16*4*512] computes
     max(wT[o', i], x[b, i]) for every (b in group, o-block, i) — the
     weight view repeats via a stride-0 dim, the bc view likewise; 32768
     free elements per instruction (the ISA num_elem field caps at 65535,
     and the fp32 scratch at 128KB/partition is the SBUF limit).
  3. One DVE tensor_reduce(min, axis=X) over the scratch viewed as
     [128, 16, 4, 512] finishes min over i, writing the [128, 16*4] result
     with a strided AP directly into the output tile.

The per-core result lands as ot[128, 4*128] = [o-within-block, block*128+b];
the host reassembles it into out[b, o]. Exact fp32 (min/max select values,
so the result is bit-identical to the reference).

This shape is chosen to be near-optimal both on real silicon (DVE-bound at
~550us/core by the calibrated cost model: TT and reduce both run 1x fp32 at
~1 elem/cycle/lane) and in instruction-dispatch-bound runtimes (only ~30
instructions per core).
"""

import os
import sys

sys.path.insert(0, "/opt/trn_rl_repo")

import numpy as np

import concourse.bass as bass
import concourse.mybir as mybir
from concourse.bass_utils import run_bass_kernel_spmd

B, I, O = 1024, 512, 512
NCORES = 8
BS = B // NCORES   # 128 batch rows per core
NCH = I // 128     # 4 i-chunks
OBLK = O // 128    # 4 output-feature blocks

# Flipped to True by test.py to collect an NTFF profile; results stashed in
# LAST_RESULTS for inspection.
TRACE = False
LAST_RESULTS = None
# When > 0, kernel() reruns the SPMD executable this many extra times and
# records per-run wall times (seconds) in BENCH_TIMES.
BENCH = 0
BENCH_TIMES = None

_F32 = mybir.dt.float32
_F16 = mybir.dt.float16

# "fp32" (exact) or "fp16" (faster DVE modes, ~1e-4 rel err)
DTYPE_MODE = os.environ.get("MINMAX_DTYPE", "fp32")


def _build_nc_wide(dt, detect_races=True, repeat=1, group=16):
    """Wide-group variant: GROUP batch rows per step, three fat instructions
    per group (DMA partition-broadcast of x rows; one wide tensor_tensor max
    over [128, GROUP*4*512]; one strided tensor_reduce min). Minimizes
    instruction count while staying near the DVE roofline.

    Needs wT = weight.T from the host: tiles wT_t[o', i] put o' on partitions
    so the i axis is free (reduce axis); x rows broadcast across partitions.
    """
    nc = bass.Bass(detect_race_conditions=detect_races)
    G = group
    NGRP = BS // G
    W = G * NCH * I  # wide op free size per group

    xd = nc.declare_dram_parameter("x", [BS, I], dt, isOutput=False)
    wt_d = nc.declare_dram_parameter("wT", [O, I], dt, isOutput=False)
    out_d = nc.declare_dram_parameter("ot", [128, OBLK * BS], dt, isOutput=True)

    x_rows = xd.rearrange("(g r) i -> g (r i)", r=G)  # [NGRP, G*I]

    with (
        nc.sbuf_tensor([128, OBLK * I], dt) as wt_sb,
        nc.sbuf_tensor([128, 2 * G * I], dt) as bc_sb,   # double-buffered bcast
        nc.sbuf_tensor([128, W], dt) as scr_sb,
        nc.sbuf_tensor([128, OBLK * BS], dt) as ot_sb,
        nc.semaphore("dma_sem") as dma_sem,
        nc.semaphore("v_sem") as v_sem,
        nc.Block() as block,
    ):
        NB = repeat * NGRP

        def bc_tile(g):
            j = g % 2
            return bc_sb[:, j * G * I:(j + 1) * G * I]

        @block.sync
        def _(sync):
            sync.dma_start(
                out=wt_sb[:, :].rearrange("p (t i) -> p t i", t=OBLK),
                in_=wt_d.rearrange("(t p) i -> p t i", p=128),
            ).then_inc(dma_sem, 16)
            # broadcast x rows two groups at a time (one DMA fills both
            # halves of the double buffer)
            n_pair_dma = 0
            for gg in range(0, NB, 2):
                g = gg % NGRP
                if gg >= 2:
                    # both halves consumed by the TTs of gg-2 and gg-1
                    sync.wait_ge(v_sem, 2 * gg - 1)
                src = x_rows[g:g + 2, :]
                src_b = bass.AP(
                    tensor=src.tensor,
                    offset=src.offset,
                    ap=[[0, 128], [G * I, 2], [1, G * I]],
                )
                sync.dma_start(out=bc_sb[:, :], in_=src_b).then_inc(dma_sem, 16)
                n_pair_dma += 1
            sync.wait_ge(v_sem, 2 * NB)
            sync.dma_start(out=out_d[:, :], in_=ot_sb[:, :]).then_inc(dma_sem, 16)
            sync.wait_ge(dma_sem, 16 * (n_pair_dma + 2))

        @block.vector
        def _(vector):
            wt_v = wt_sb[:, :]
            scr_v = scr_sb[:, :]
            for gg in range(NB):
                g = gg % NGRP
                if gg % 2 == 0:
                    vector.wait_ge(dma_sem, 16 * (gg // 2 + 2))
                bc = bc_tile(gg)
                in0 = bass.AP(
                    tensor=wt_v.tensor, offset=wt_v.offset,
                    ap=[[wt_v.ap[0][0], 128], [0, G], [I, OBLK], [1, I]],
                )
                in1 = bass.AP(
                    tensor=bc.tensor, offset=bc.offset,
                    ap=[[bc.ap[0][0], 128], [I, G], [0, OBLK], [1, I]],
                )
                out = bass.AP(
                    tensor=scr_v.tensor, offset=scr_v.offset,
                    ap=[[scr_v.ap[0][0], 128], [OBLK * I, G], [I, OBLK], [1, I]],
                )
                nc.vector.tensor_tensor(
                    out=out, in0=in0, in1=in1, op=mybir.AluOpType.max
                ).then_inc(v_sem, 1)
                ot_ap = ot_sb[:, :]
                red_out = bass.AP(
                    tensor=ot_ap.tensor,
                    offset=ot_ap.offset + g * G,
                    ap=[[ot_ap.ap[0][0], 128], [1, G], [BS, OBLK]],
                )
                nc.vector.tensor_reduce(
                    out=red_out,
                    in_=out,
                    op=mybir.AluOpType.min,
                    axis=mybir.AxisListType.X,
                ).then_inc(v_sem, 1)

    return nc


_NC_CACHE = {}


def _get_nc(mode):
    if mode not in _NC_CACHE:
        if mode == "fp16":
            _NC_CACHE[mode] = _build_nc_wide(_F16, group=16)
        else:
            _NC_CACHE[mode] = _build_nc_wide(_F32, group=16)
    return _NC_CACHE[mode]


def kernel(x, weight):
    global LAST_RESULTS
    x = np.asarray(x)
    weight = np.asarray(weight)
    in_dtype = x.dtype

    mode = DTYPE_MODE
    npdt = np.float16 if mode == "fp16" else np.float32
    nc = _get_nc(mode)

    wt_h = np.ascontiguousarray(weight.T.astype(npdt))  # [O, I]
    xh = x.astype(npdt)
    in_maps = [
        {
            "x": np.ascontiguousarray(xh[c * BS:(c + 1) * BS]),
            "wT": wt_h,
        }
        for c in range(NCORES)
    ]

    res = run_bass_kernel_spmd(nc, in_maps, list(range(NCORES)), trace=TRACE)
    LAST_RESULTS = res

    if BENCH > 0:
        import time as _time

        global BENCH_TIMES
        BENCH_TIMES = []
        for _ in range(BENCH):
            t0 = _time.perf_counter()
            run_bass_kernel_spmd(nc, in_maps, list(range(NCORES)), trace=False)
            BENCH_TIMES.append(_time.perf_counter() - t0)

    # ot[oo, t*BS + b] = out_core[b, t*128 + oo]
    parts = []
    for c in range(NCORES):
        ot = np.asarray(res.results[c]["ot"])          # [128, OBLK*BS]
        oc = ot.reshape(128, OBLK, BS).transpose(2, 1, 0).reshape(BS, O)
        parts.append(oc)
    out = np.concatenate(parts, axis=0)
    return out.astype(in_dtype)


# revision 38
# speedup vs baseline: 346.5315x; 1.0168x over previous
"""Tropical min-max matmul kernel for Trainium2.

out[b, o] = min_i max(x[b, i], weight[i, o])   with  x: [1024, 512], weight: [512, 512], fp32.

Strategy
--------
Data-parallel over the batch dim: 8 NeuronCores x 128 rows of x each; weight
replicated (no collectives). Per core, the weight is held transposed
(wT[o, i], o on partitions in 4 row-blocks) so the contraction axis i is the
DVE free axis, and batch rows are processed in groups of 16 with three fat
instructions per group:

  1. A DMA whose source access pattern has partition stride 0 broadcasts the
     group's x rows across all 128 partitions (SBUF bc tile, double-buffered,
     two groups loaded per DMA).
  2. One wide DVE tensor_tensor(max) over [128, 16*4*512] computes
     max(wT[o', i], x[b, i]) for every (b in group, o-block, i) — the
     weight view repeats via a stride-0 dim, the bc view likewise; 32768
     free elements per instruction (the ISA num_elem field caps at 65535,
     and the fp32 scratch at 128KB/partition is the SBUF limit).
  3. One DVE tensor_reduce(min, axis=X) over the scratch viewed as
     [128, 16, 4, 512] finishes min over i, writing the [128, 16*4] result
     with a strided AP directly into the output tile.

The per-core result lands as ot[128, 4*128] = [o-within-block, block*128+b];
the host reassembles it into out[b, o]. Exact fp32 (min/max select values,
so the result is bit-identical to the reference).

This shape is chosen to be near-optimal both on real silicon (DVE-bound at
~550us/core by the calibrated cost model: TT and reduce both run 1x fp32 at
~1 elem/cycle/lane) and in instruction-dispatch-bound runtimes (only ~30
instructions per core).
"""

import os
import sys

sys.path.insert(0, "/opt/trn_rl_repo")

import numpy as np

import concourse.bass as bass
import concourse.mybir as mybir
from concourse.bass_utils import run_bass_kernel_spmd

B, I, O = 1024, 512, 512
NCORES = 8
BS = B // NCORES   # 128 batch rows per core
NCH = I // 128     # 4 i-chunks
OBLK = O // 128    # 4 output-feature blocks

# Flipped to True by test.py to collect an NTFF profile; results stashed in
# LAST_RESULTS for inspection.
TRACE = False
LAST_RESULTS = None
# When > 0, kernel() reruns the SPMD executable this many extra times and
# records per-run wall times (seconds) in BENCH_TIMES.
BENCH = 0
BENCH_TIMES = None

_F32 = mybir.dt.float32
_F16 = mybir.dt.float16

# "fp32" (exact) or "fp16" (faster DVE modes, ~1e-4 rel err)
DTYPE_MODE = os.environ.get("MINMAX_DTYPE", "fp32")


def _build_nc_wide(dt, detect_races=True, repeat=1, group=16):
    """Wide-group variant: GROUP batch rows per step, three fat instructions
    per group (DMA partition-broadcast of x rows; one wide tensor_tensor max
    over [128, GROUP*4*512]; one strided tensor_reduce min). Minimizes
    instruction count while staying near the DVE roofline.

    Needs wT = weight.T from the host: tiles wT_t[o', i] put o' on partitions
    so the i axis is free (reduce axis); x rows broadcast across partitions.
    """
    nc = bass.Bass(detect_race_conditions=detect_races)
    G = group
    NGRP = BS // G
    W = G * NCH * I  # wide op free size per group

    xd = nc.declare_dram_parameter("x", [BS, I], dt, isOutput=False)
    wt_d = nc.declare_dram_parameter("wT", [O, I], dt, isOutput=False)
    out_d = nc.declare_dram_parameter("ot", [128, OBLK * BS], dt, isOutput=True)

    x_rows = xd.rearrange("(g r) i -> g (r i)", r=G)  # [NGRP, G*I]

    with (
        nc.sbuf_tensor([128, OBLK * I], dt) as wt_sb,
        nc.sbuf_tensor([128, 2 * G * I], dt) as bc_sb,   # double-buffered bcast
        nc.sbuf_tensor([128, W], dt) as scr_sb,
        nc.sbuf_tensor([128, OBLK * BS], dt) as ot_sb,
        nc.semaphore("dma_sem") as dma_sem,
        nc.semaphore("v_sem") as v_sem,
        nc.Block() as block,
    ):
        NB = repeat * NGRP

        def bc_tile(g):
            j = g % 2
            return bc_sb[:, j * G * I:(j + 1) * G * I]

        @block.sync
        def _(sync):
            sync.dma_start(
                out=wt_sb[:, :].rearrange("p (t i) -> p t i", t=OBLK),
                in_=wt_d.rearrange("(t p) i -> p t i", p=128),
            ).then_inc(dma_sem, 16)
            # broadcast x rows two groups at a time (one DMA fills both
            # halves of the double buffer)
            n_pair_dma = 0
            for gg in range(0, NB, 2):
                g = gg % NGRP
                if gg >= 2:
                    # both halves consumed by the TTs of gg-2 and gg-1
                    sync.wait_ge(v_sem, 2 * gg - 1)
                src = x_rows[g:g + 2, :]
                src_b = bass.AP(
                    tensor=src.tensor,
                    offset=src.offset,
                    ap=[[0, 128], [G * I, 2], [1, G * I]],
                )
                sync.dma_start(out=bc_sb[:, :], in_=src_b).then_inc(dma_sem, 16)
                n_pair_dma += 1
            sync.wait_ge(v_sem, 2 * NB)
            sync.dma_start(out=out_d[:, :], in_=ot_sb[:, :]).then_inc(dma_sem, 16)
            sync.wait_ge(dma_sem, 16 * (n_pair_dma + 2))

        @block.vector
        def _(vector):
            wt_v = wt_sb[:, :]
            scr_v = scr_sb[:, :]
            for gg in range(NB):
                g = gg % NGRP
                if gg % 2 == 0:
                    vector.wait_ge(dma_sem, 16 * (gg // 2 + 2))
                bc = bc_tile(gg)
                in0 = bass.AP(
                    tensor=wt_v.tensor, offset=wt_v.offset,
                    ap=[[wt_v.ap[0][0], 128], [0, G], [I, OBLK], [1, I]],
                )
                in1 = bass.AP(
                    tensor=bc.tensor, offset=bc.offset,
                    ap=[[bc.ap[0][0], 128], [I, G], [0, OBLK], [1, I]],
                )
                out = bass.AP(
                    tensor=scr_v.tensor, offset=scr_v.offset,
                    ap=[[scr_v.ap[0][0], 128], [OBLK * I, G], [I, OBLK], [1, I]],
                )
                nc.vector.tensor_tensor(
                    out=out, in0=in0, in1=in1, op=mybir.AluOpType.max
                ).then_inc(v_sem, 1)
                ot_ap = ot_sb[:, :]
                red_out = bass.AP(
                    tensor=ot_ap.tensor,
                    offset=ot_ap.offset + g * G,
                    ap=[[ot_ap.ap[0][0], 128], [1, G], [BS, OBLK]],
                )
                nc.vector.tensor_reduce(
                    out=red_out,
                    in_=out,
                    op=mybir.AluOpType.min,
                    axis=mybir.AxisListType.X,
                ).then_inc(v_sem, 1)

    return nc


_NC_CACHE = {}


def _get_nc(mode):
    if mode not in _NC_CACHE:
        if mode == "fp16":
            _NC_CACHE[mode] = _build_nc_wide(_F16, group=16)
        else:
            _NC_CACHE[mode] = _build_nc_wide(_F32, group=16)
    return _NC_CACHE[mode]


def kernel(x, weight):
    global LAST_RESULTS
    x = np.asarray(x)
    weight = np.asarray(weight)
    in_dtype = x.dtype

    mode = DTYPE_MODE
    npdt = np.float16 if mode == "fp16" else np.float32
    nc = _get_nc(mode)

    wt_h = np.ascontiguousarray(weight.T.astype(npdt))  # [O, I]
    xh = x.astype(npdt)
    in_maps = [
        {
            "x": np.ascontiguousarray(xh[c * BS:(c + 1) * BS]),
            "wT": wt_h,
        }
        for c in range(NCORES)
    ]

    res = run_bass_kernel_spmd(nc, in_maps, list(range(NCORES)), trace=TRACE)
    LAST_RESULTS = res

    if BENCH > 0:
        import time as _time

        global BENCH_TIMES
        BENCH_TIMES = []
        for _ in range(BENCH):
            t0 = _time.perf_counter()
            run_bass_kernel_spmd(nc, in_maps, list(range(NCORES)), trace=False)
            BENCH_TIMES.append(_time.perf_counter() - t0)

    # ot[oo, t*BS + b] = out_core[b, t*128 + oo]
    parts = []
    for c in range(NCORES):
        ot = np.asarray(res.results[c]["ot"])          # [128, OBLK*BS]
        oc = ot.reshape(128, OBLK, BS).transpose(2, 1, 0).reshape(BS, O)
        parts.append(oc)
    out = np.concatenate(parts, axis=0)
    return out.astype(in_dtype)
